# revision 23
# baseline (speedup 1.0000x reference)
"""Trainium2 Bass kernel for the CustomExtractorSNN forward pass.

Strategy
--------
Pure data parallel over 8 NeuronCores: the batch dim of x is split into 8
shards; the tiny weights / beta / threshold vectors are replicated.

Device layout is feature-major ("transposed"): the host passes x already
transposed ([256, B_core]) so every DMA is contiguous and the device needs
zero transposes.  Activations live as [feature, batch] tiles.  Two batch
chunks of FD columns are stacked on the 128 SBUF partitions (features 0-63
= chunk A, 64-127 = chunk B) so elementwise engines run at full width;
matmuls use block-diagonal weights to map stacked -> stacked.

Math (after dead-code elimination of the reference):
  Only mem1 is recurrent.  With scaled state M = mem1/thr, c = cur1/thr:
      M_t = beta*M_{t-1} + c - spk_{t-1},   spk_t = (M_t > 1)
  (reset_t == spk_{t-1} because lif2's state is mem1 and reset is computed
  from the pre-update membrane.)  mem2/mem3 only matter at the final step:
      M2 = beta*M_10 + (cur2raw + b2)/thr - spk_10
      mem2 = M2*thr ; spk2 = (M2 > 1)
      mem3 = clip(beta_out)*mem2 + (spk2 @ Wo.T + bo)
      actor = tanh(mem3) * pi
      critic = relu(relu(x @ Wv1.T + bv1) @ Wv2.T + bv2)
"""

import os
from contextlib import ExitStack

import numpy as np

import concourse.bass as bass
import concourse.tile as tile
from concourse import bacc, mybir
from concourse.bass_utils import run_bass_kernel_spmd

N_CORES = 8
B_FULL = 131072
F = 256  # input features
H = 64   # hidden (= A = V = 64)
B_CORE = B_FULL // N_CORES

FD = 1024          # free-dim (batch columns) per chunk; 2 PSUM banks
CHUNK = 2 * FD     # batch columns per supertile (stacked pair)
TIMESTEPS = 10
PI = float(np.pi)
WPACK_COLS = 512 + 512 + 128 * 3 + 9  # 1417

f32 = mybir.dt.float32
Alu = mybir.AluOpType
Act = mybir.ActivationFunctionType

_BUILD_CACHE: dict = {}


def _build(bcore: int) -> bass.Bass:
    """Build the single-core Bass program (same program runs SPMD on all cores)."""
    if bcore in _BUILD_CACHE:
        return _BUILD_CACHE[bcore]
    assert bcore % CHUNK == 0
    n_super = bcore // CHUNK
    nj = FD // 512  # matmul N-splits per chunk

    nc = bacc.Bacc(
        "TRN2", target_bir_lowering=False, debug=False, num_devices=N_CORES
    )

    xT = nc.dram_tensor("xT", [F, bcore], f32, kind="ExternalInput")
    # all weights/constants packed column-wise into one tensor:
    # [0:512] w1 chunks, [512:1024] wv1 chunks, [1024:1152] w2bd,
    # [1152:1280] wobd, [1280:1408] wv2bd, [1408:1417] vecs
    # vec columns: 0: 1/thr  1: beta  2: thr  3: b1/thr  4: b2/thr  5: bo
    #              6: bv1  7: bv2  8: clip(beta_out)
    wpack = nc.dram_tensor("wpack", [128, WPACK_COLS], f32, kind="ExternalInput")

    actorT = nc.dram_tensor("actorT", [H, bcore], f32, kind="ExternalOutput")
    criticT = nc.dram_tensor("criticT", [H, bcore], f32, kind="ExternalOutput")

    with tile.TileContext(nc) as tc, ExitStack() as ctx:
        wpool = ctx.enter_context(tc.tile_pool(name="weights", bufs=1))
        xpool = ctx.enter_context(tc.tile_pool(name="x", bufs=3))
        mpool = ctx.enter_context(tc.tile_pool(name="work", bufs=2))
        opool = ctx.enter_context(tc.tile_pool(name="outs", bufs=3))
        ps_c1 = ctx.enter_context(
            tc.tile_pool(name="ps_c1", bufs=2, space=bass.MemorySpace.PSUM)
        )
        ps_v1 = ctx.enter_context(
            tc.tile_pool(name="ps_v1", bufs=1, space=bass.MemorySpace.PSUM)
        )
        ps_mm = ctx.enter_context(
            tc.tile_pool(name="ps_mm", bufs=1, space=bass.MemorySpace.PSUM)
        )

        # Load weights, then re-materialize via VectorE so that every matmul's
        # weight dependency is a DVE-engine sem, not a second DMA-queue sem
        # (LDWEIGHTS has a tiny sync-wait budget).
        wld = wpool.tile([128, WPACK_COLS], f32, tag="wld")
        nc.sync.dma_start(wld[:], wpack[:])
        wall = wpool.tile([128, WPACK_COLS], f32, tag="wpack")
        nc.vector.tensor_copy(wall[:], wld[:])
        # Dummy 1-col matmul: makes PE observe the DVE weight-copy tick once,
        # so real matmuls don't need a second (DVE) sync wait on top of their
        # x-DMA wait (LDWEIGHTS has a 1-wait budget).
        warm = ps_mm.tile([1, 1], f32, tag="mm2")
        nc.tensor.matmul(warm[:], wall[:, 0:1], wall[:, 0:1], start=True, stop=True)

        w1 = wall[:, 0:512]
        wv1 = wall[:, 512:1024]
        w2 = wall[:, 1024:1152]
        wo = wall[:, 1152:1280]
        wv2 = wall[:, 1280:1408]
        vec = wall[:, 1408:1417]

        invthr = vec[:, 0:1]
        beta = vec[:, 1:2]
        thr = vec[:, 2:3]
        c1b = vec[:, 3:4]
        c2b = vec[:, 4:5]
        bo = vec[:, 5:6]
        bv1 = vec[:, 6:7]
        bv2 = vec[:, 7:8]
        bov = vec[:, 8:9]

        for s in range(n_super):
            a0 = s * CHUNK
            # ---- load xT: one simple 2D DMA per K-chunk ----
            # k%2 = feature half, k//2 = batch chunk
            xt = []
            for k in range(4):
                t = xpool.tile([128, FD], f32, tag=f"x{k}")
                half = (k % 2) * 128
                col = a0 + (k // 2) * FD
                nc.sync.dma_start(t[:], xT[half : half + 128, col : col + FD])
                xt.append(t)

            def xs(k, j):  # rhs slice for K-chunk k, N-split j
                return xt[k][:, j * 512 : (j + 1) * 512]

            # ---- cur1 = blockdiag(W1) @ xT  (K=512 via 4 accumulating MMs) ----
            c1ps = ps_c1.tile([128, FD], f32, tag="c1")
            for j in range(nj):
                for k in range(4):
                    nc.tensor.matmul(
                        c1ps[:, j * 512 : (j + 1) * 512],
                        w1[:, k * 128 : (k + 1) * 128],
                        xs(k, j),
                        start=(k == 0),
                        stop=(k == 3),
                    )
            # c1p = cur1/thr + b1/thr  (on DVE so the psum release is a DVE dep)
            c1p = mpool.tile([128, FD], f32, tag="c1p")
            nc.vector.tensor_scalar(c1p[:], c1ps[:], invthr, c1b, Alu.mult, Alu.add)

            # ---- critic path ----
            v1ps = ps_v1.tile([128, FD], f32, tag="v1")
            for j in range(nj):
                for k in range(4):
                    nc.tensor.matmul(
                        v1ps[:, j * 512 : (j + 1) * 512],
                        wv1[:, k * 128 : (k + 1) * 128],
                        xs(k, j),
                        start=(k == 0),
                        stop=(k == 3),
                    )
            v1 = mpool.tile([128, FD], f32, tag="v1s")
            nc.vector.tensor_scalar(v1[:], v1ps[:], bv1, 0.0, Alu.add, Alu.max)
            v2ps = ps_mm.tile([128, FD], f32, tag="mm2")
            for j in range(nj):
                nc.tensor.matmul(
                    v2ps[:, j * 512 : (j + 1) * 512],
                    wv2[:],
                    v1[:, j * 512 : (j + 1) * 512],
                    start=True,
                    stop=True,
                )
            critic = opool.tile([128, FD], f32, tag="critic")
            nc.vector.tensor_scalar(critic[:], v2ps[:], bv2, 0.0, Alu.add, Alu.max)
            nc.sync.dma_start(criticT[:, a0 : a0 + FD], critic[0:64, :])
            nc.sync.dma_start(criticT[:, a0 + FD : a0 + CHUNK], critic[64:128, :])

            # ---- LIF recurrence: M_t = beta*M - ((M>1) - c1p) ----
            M = mpool.tile([128, FD], f32, tag="M")
            nc.vector.tensor_copy(M[:], c1p[:])  # M_1 = c1p
            w = mpool.tile([128, FD], f32, tag="w")
            for _ in range(TIMESTEPS - 1):
                nc.vector.scalar_tensor_tensor(
                    w[:], M[:], 1.0, c1p[:], Alu.is_gt, Alu.subtract
                )
                nc.vector.scalar_tensor_tensor(
                    M[:], M[:], beta, w[:], Alu.mult, Alu.subtract
                )

            # ---- final-step lif2 / lif3 chain ----
            spk = mpool.tile([128, FD], f32, tag="spk")
            nc.vector.tensor_scalar(spk[:], M[:], 1.0, None, Alu.is_gt)
            u = mpool.tile([128, FD], f32, tag="u")
            nc.vector.tensor_scalar(u[:], M[:], beta, c2b, Alu.mult, Alu.add)
            nc.vector.tensor_sub(u[:], u[:], spk[:])
            c2ps = ps_mm.tile([128, FD], f32, tag="mm2")
            for j in range(nj):
                nc.tensor.matmul(
                    c2ps[:, j * 512 : (j + 1) * 512],
                    w2[:],
                    spk[:, j * 512 : (j + 1) * 512],
                    start=True,
                    stop=True,
                )
            M2 = mpool.tile([128, FD], f32, tag="M2")
            nc.vector.scalar_tensor_tensor(
                M2[:], c2ps[:], invthr, u[:], Alu.mult, Alu.add
            )
            spk2 = mpool.tile([128, FD], f32, tag="spk2")
            nc.vector.tensor_scalar(spk2[:], M2[:], 1.0, None, Alu.is_gt)
            mem2 = mpool.tile([128, FD], f32, tag="mem2")
            nc.vector.tensor_scalar(mem2[:], M2[:], thr, None, Alu.mult)
            c3ps = ps_mm.tile([128, FD], f32, tag="mm2")
            for j in range(nj):
                nc.tensor.matmul(
                    c3ps[:, j * 512 : (j + 1) * 512],
                    wo[:],
                    spk2[:, j * 512 : (j + 1) * 512],
                    start=True,
                    stop=True,
                )
            m3 = mpool.tile([128, FD], f32, tag="m3")
            nc.vector.scalar_tensor_tensor(
                m3[:], mem2[:], bov, c3ps[:], Alu.mult, Alu.add
            )
            act = opool.tile([128, FD], f32, tag="act")
            nc.scalar.activation(act[:], m3[:], Act.Tanh, bias=bo, scale=1.0)
            nc.vector.tensor_scalar(act[:], act[:], PI, None, Alu.mult)
            nc.sync.dma_start(actorT[:, a0 : a0 + FD], act[0:64, :])
            nc.sync.dma_start(actorT[:, a0 + FD : a0 + CHUNK], act[64:128, :])

    nc.finalize()
    _BUILD_CACHE[bcore] = nc
    return nc


def _blockdiag2(w: np.ndarray) -> np.ndarray:
    """[[w, 0], [0, w]] for a 64x64 w -> 128x128."""
    out = np.zeros((128, 128), np.float32)
    out[0:64, 0:64] = w
    out[64:128, 64:128] = w
    return out


def _make_consts(W1, b1, W2, b2, Wo, bo, beta_in, thr_in, beta_out, Wv1, bv1, Wv2, bv2):
    # lhsT chunks for cur1: out = lhsT.T @ rhs. Chunk A -> out partitions
    # 0:64, chunk B -> 64:128.  k0/k1: features 0:128 / 128:256 for chunk A.
    def chunks(W):  # W: [64, 256] -> [128, 4*128], chunk k at cols k*128:(k+1)*128
        c = np.zeros((4, 128, 128), np.float32)
        c[0, :, 0:64] = W[:, 0:128].T
        c[1, :, 0:64] = W[:, 128:256].T
        c[2, :, 64:128] = W[:, 0:128].T
        c[3, :, 64:128] = W[:, 128:256].T
        return np.ascontiguousarray(np.concatenate(list(c), axis=1))

    beta_c = np.clip(beta_in, 0.0, 1.0).astype(np.float32)
    thr = thr_in.astype(np.float32)
    invthr = (np.float32(1.0) / thr).astype(np.float32)
    bo_clip = np.float32(np.clip(beta_out, 0.0, 1.0)[0])

    def st(v):  # stack a [64] vector to [128]
        return np.tile(v.astype(np.float32), 2)

    vecs = np.stack(
        [
            st(invthr),
            st(beta_c),
            st(thr),
            st(b1 * invthr),
            st(b2 * invthr),
            st(bo),
            st(bv1),
            st(bv2),
            np.full(128, bo_clip, np.float32),
        ],
        axis=1,
    ).astype(np.float32)

    wpack = np.concatenate(
        [
            chunks(W1),
            chunks(Wv1),
            _blockdiag2(W2.T),
            _blockdiag2(Wo.T),
            _blockdiag2(Wv2.T),
            vecs,
        ],
        axis=1,
    )
    assert wpack.shape == (128, WPACK_COLS)
    return dict(wpack=np.ascontiguousarray(wpack))


def _run(x, consts, bcore):
    nc = _build(bcore)
    n_cores = x.shape[0] // bcore
    xTf = np.ascontiguousarray(x.T.astype(np.float32))  # [256, B]
    in_maps = []
    for c in range(n_cores):
        m = dict(consts)
        m["xT"] = np.ascontiguousarray(xTf[:, c * bcore : (c + 1) * bcore])
        in_maps.append(m)
    res = run_bass_kernel_spmd(nc, in_maps, list(range(n_cores)))
    actorT = np.concatenate([r["actorT"] for r in res.results], axis=1)
    criticT = np.concatenate([r["criticT"] for r in res.results], axis=1)
    actor = np.ascontiguousarray(actorT.T)
    critic = np.ascontiguousarray(criticT.T)
    return actor, critic


def kernel(x, W1, b1, W2, b2, Wo, bo, beta_in, thr_in, beta_out, Wv1, bv1, Wv2, bv2):
    x = np.asarray(x, np.float32)
    consts = _make_consts(
        np.asarray(W1, np.float32), np.asarray(b1, np.float32),
        np.asarray(W2, np.float32), np.asarray(b2, np.float32),
        np.asarray(Wo, np.float32), np.asarray(bo, np.float32),
        np.asarray(beta_in, np.float32), np.asarray(thr_in, np.float32),
        np.asarray(beta_out, np.float32),
        np.asarray(Wv1, np.float32), np.asarray(bv1, np.float32),
        np.asarray(Wv2, np.float32), np.asarray(bv2, np.float32),
    )
    return _run(x, consts, B_CORE)
